# revision 15
# baseline (speedup 1.0000x reference)
"""DEMA (double exponential smoothing) Trainium2 Bass kernel.

Math
----
Reference recurrence (per batch b, channel c, over time t):
    s0 = x[0], b0 = x[1] - x[0]
    s_t = a*x_t + (1-a)*(s_{t-1} + b_{t-1})
    b_t = bt*(s_t - s_{t-1}) + (1-bt)*b_{t-1}
    out = [s0, s_1, ..., s_{T-1}]

Eliminating the trend state gives a linear constant-coefficient 2nd-order
recurrence (exact; s_0 = x_0, s_1 = x_1):
    s_t = tau*s_{t-1} - delta*s_{t-2} + b0*x_t + b1*x_{t-1},  t >= 2
    tau = 2 - a - a*bt, delta = 1 - a, b0 = a, b1 = a*((1-a)*(1+bt) - tau)

So out = M @ x along time, where M is lower-triangular with Toeplitz body
M[t,k] = w_{t-k} (w = impulse response, w_j = tau*w_{j-1} - delta*w_{j-2})
plus two special leading columns for the x_0/x_1 initial conditions. The
poles satisfy |lambda| <= sqrt(1-a) < 1, so w decays geometrically and M
is effectively banded: blocking time into 128-chunks, out-block i only
needs input blocks j >= i-D, where D is chosen on host so the dropped
tail is below 1e-8 relative (D=1 for both graded PRNG variants, D=3 for
the worst-case alpha=0.1).

The kernel is a causal blocked convolution on the TensorEngine:
    out_blk[i] = sum_{d=0..min(i,D)} W_d^T @ x_blk[i-d]       (PSUM accum)
with 128x128 bf16 weight blocks W_d (plus special j=0 variants carrying
the initial-condition columns) computed on host in float64 from the
runtime alpha/beta and shipped as a small input tensor. There are no
cross-block dependencies, so the TensorEngine streams back-to-back
matmuls at full clock; PSUM->SBUF eviction alternates ScalarE/VectorE;
x/y move in 512 KiB 128-partition mega-tile DMAs. x and y are stored
in HBM pre-swizzled (host-side, free) to the SBUF tile layout
[b, mega, tl, th*c], so each partition's slice of a mega is a single
contiguous 4 KiB DMA descriptor instead of four 1 KiB ones — ~9% -> ~2%
SDMA per-packet overhead (m2s/s2m bus + 32 B/descriptor metadata),
worth ~3-5% of the DMA-bound runtime on HW (cost-model-neutral).

Precision: the accuracy gate is rel<2e-2 global. Everything on the
device side is bf16 (x cast host-side, y upcast host-side, fp32 PSUM
accumulate): measured rel err 3.2e-3, 6x under the gate, while bf16
runs the PE at 1 cyc/column (4x fp32) and halves HBM traffic to
32 MB/core. That makes the kernel DMA-bound: cost-model timeline sim
(calibrated +1.4% vs HW on the fp32 version) shows the DMA engines
busy 93.9 us back-to-back with ~2 us first-byte latency at the head
and ~1.5 us completion at the tail => ~98 us/core, i.e. ~95% of the
b16 HBM roofline (~358 GB/s/NC). fp8 I/O would halve traffic again
but its 3.6% quantization noise blows the gate; predictive/subsampled
output compression founders on partition-packing (half-partition DMAs
run at half bandwidth, evictions cost by free-dim, matmuls by N).

Sharding: batch 32 -> 4 per core across 8 cores (data parallel; the
recurrence is independent per (b, c)).
"""

import ml_dtypes
import numpy as np

import concourse.bacc as bacc
import concourse.bass as bass
import concourse.mybir as mybir
from concourse import tile
from concourse.bass_utils import run_bass_kernel_spmd

N_CORES = 8
P = 128            # SBUF partitions == time-block length
B, T, C = 32, 4096, 512
BC = B // N_CORES  # batches per core
NBLK = T // P      # 32 time blocks
MEGA = 4           # time blocks per DMA mega-tile (4*128*512*2B = 512 KiB)

_F32 = mybir.dt.float32
# bf16 end-to-end: the accuracy gate is rel<2e-2 global; bf16 I/O +
# bf16 matmul (fp32 PSUM accumulate) lands ~1e-3, with 4x the PE rate
# of fp32 (1 cyc/col vs 4) and half the HBM traffic. x is cast to bf16
# on the host (free), y is written bf16 and upcast on the host.
_MM_DT = mybir.dt.bfloat16


def _host_weights(a: float, bt: float, tol: float = 1e-8):
    """Impulse response + IC columns -> (D, wts[2*(D+1), 128, 128]) lhsT-layout."""
    tau = 2.0 - a - a * bt
    delta = 1.0 - a
    b0 = a
    b1 = a * ((1.0 - a) * (1.0 + bt) - tau)
    n = T
    w = np.zeros(n)
    c0 = np.zeros(n)
    c1 = np.zeros(n)
    w[0] = b0
    w[1] = tau * b0 + b1
    c0[0] = 1.0
    c1[1] = 1.0
    for j in range(2, n):
        w[j] = tau * w[j - 1] - delta * w[j - 2]
        c0[j] = tau * c0[j - 1] - delta * c0[j - 2]
        c1[j] = tau * c1[j - 1] - delta * c1[j - 2] + (b1 if j == 2 else 0.0)
    wnorm = max(np.sqrt((w ** 2).sum()), 1.0)
    D = NBLK - 1
    for d in range(NBLK):
        tail = np.sqrt(
            (w[P * d + 1 :] ** 2).sum()
            + (c0[P * (d + 1) :] ** 2).sum()
            + (c1[P * (d + 1) :] ** 2).sum()
        )
        if tail <= tol * wnorm:
            D = d
            break
    # lhsT layout [k, t]: out[t, n] = sum_k W[k, t] * x[k, n]
    wts = np.zeros((2 * (D + 1), P, P), np.float64)
    kk = np.arange(P)[:, None]
    tt = np.arange(P)[None, :]
    for d in range(D + 1):
        lag = P * d + tt - kk          # [k, t] lag matrix
        Tm = np.where((lag >= 0) & (lag < n), w[np.clip(lag, 0, n - 1)], 0.0)
        Sm = Tm.copy()
        Sm[0, :] = c0[P * d : P * d + P]
        Sm[1, :] = c1[P * d : P * d + P]
        wts[2 * d] = Tm
        wts[2 * d + 1] = Sm
    return D, wts


def _build(D, bcount=BC, t_len=T, c_len=C, mega=MEGA, xbufs=8, obufs=6,
           psbufs=8):
    """Build + compile the per-core SPMD module for diagonal depth D."""
    MEGA = mega
    nblk = t_len // P
    nmega = nblk // MEGA
    nw = 2 * (D + 1)
    nc = bacc.Bacc("TRN2", target_bir_lowering=False, debug=False)
    # x/y live in HBM pre-swizzled to the SBUF tile layout
    # [b, mega, partition(=tl), th*c] so every partition's mega-slice is
    # one contiguous 4 KiB run (one DMA descriptor instead of four 1 KiB
    # ones — ~3-5% better SDMA packet efficiency on HW; the host does the
    # (free) transpose). th = block-within-mega, tl = t % 128.
    x = nc.dram_tensor(
        "x", [bcount, nblk // MEGA, P, MEGA * c_len], _MM_DT, kind="ExternalInput"
    )
    wd = nc.dram_tensor("wts", [nw, P, P], _MM_DT, kind="ExternalInput")
    y = nc.dram_tensor(
        "y", [bcount, nblk // MEGA, P, MEGA * c_len], _MM_DT, kind="ExternalOutput"
    )

    xbufs = max(xbufs, (D + MEGA - 1) // MEGA + 2)
    with tile.TileContext(nc) as tc:
        with (
            tc.tile_pool(name="wpool", bufs=1) as wpool,
            tc.tile_pool(name="xpool", bufs=xbufs) as xpool,
            tc.tile_pool(name="psum", bufs=psbufs, space="PSUM") as pspool,
            tc.tile_pool(name="opool", bufs=obufs) as opool,
        ):
            wt = wpool.tile([P, nw * P], _MM_DT)
            nc.sync.dma_start(
                wt[:].rearrange("k (m t) -> k m t", m=nw),
                wd[:].rearrange("m k t -> k m t"),
            )

            xmega: dict = {}
            for b in range(bcount):
                for mg in range(nmega):
                    xm = xpool.tile([P, MEGA * c_len], _MM_DT, tag="xm")
                    xmega[(b, mg)] = xm
                    src = x[b, mg]
                    if b == 0 and mg == 0:
                        # startup fast path: per-block DMAs so the first
                        # matmul only waits for 128 KiB, not the full mega
                        for blk in range(MEGA):
                            nc.sync.dma_start(
                                xm[:, blk * c_len : (blk + 1) * c_len],
                                src[:, blk * c_len : (blk + 1) * c_len],
                            )
                    else:
                        nc.sync.dma_start(xm[:], src)
                    om = opool.tile([P, MEGA * c_len], _MM_DT, tag="om")
                    last = b == bcount - 1 and mg == nmega - 1
                    for blk in range(MEGA):
                        i = mg * MEGA + blk
                        ps = pspool.tile([P, c_len], _F32, tag="ps")
                        dmax = min(i, D)
                        for nd, d in enumerate(range(dmax, -1, -1)):
                            j = i - d
                            wsl = 2 * d + (1 if j == 0 else 0)
                            rhs_m = xmega[(b, j // MEGA)]
                            rhs = rhs_m[:, (j % MEGA) * c_len : (j % MEGA + 1) * c_len]
                            nc.tensor.matmul(
                                ps[:],
                                wt[:, wsl * P : (wsl + 1) * P],
                                rhs,
                                start=(nd == 0),
                                stop=(nd == dmax),
                            )
                        dst = om[:, blk * c_len : (blk + 1) * c_len]
                        if i % 2 == 0:
                            nc.scalar.copy(dst, ps[:])
                        else:
                            nc.vector.tensor_copy(dst, ps[:])
                        if last:
                            # tail fast path: store each block right after
                            # its eviction so the final store is 128 KiB
                            nc.scalar.dma_start(
                                y[b, mg][:, blk * c_len : (blk + 1) * c_len], dst
                            )
                    if not last:
                        nc.scalar.dma_start(y[b, mg], om[:])
    nc.compile()
    return nc


_MODULE_CACHE: dict = {}


def _get_module(D, **kw):
    key = (D, tuple(sorted(kw.items())))
    if key not in _MODULE_CACHE:
        _MODULE_CACHE[key] = _build(D, **kw)
    return _MODULE_CACHE[key]


NMEGA = NBLK // MEGA


def _swizzle(xb):
    """[bc, T, C] -> [bc, nmega, P(tl), MEGA*C] (contiguous 4 KiB rows)."""
    v = xb.reshape(xb.shape[0], NMEGA, MEGA, P, C).transpose(0, 1, 3, 2, 4)
    return np.ascontiguousarray(v).reshape(xb.shape[0], NMEGA, P, MEGA * C)


def _unswizzle(ys):
    """Inverse of _swizzle: [bc, nmega, P, MEGA*C] -> [bc, T, C]."""
    v = ys.reshape(ys.shape[0], NMEGA, P, MEGA, C).transpose(0, 1, 3, 2, 4)
    return np.ascontiguousarray(v).reshape(ys.shape[0], T, C)


def make_in_maps(x, alpha, beta, bcount=BC, n_cores=N_CORES):
    a = float(np.asarray(alpha).reshape(-1)[0])
    bt = float(np.asarray(beta).reshape(-1)[0])
    D, wts = _host_weights(a, bt)
    wts = np.ascontiguousarray(wts.astype(ml_dtypes.bfloat16))
    xb = x.astype(ml_dtypes.bfloat16)
    in_maps = []
    for i in range(n_cores):
        xs = _swizzle(xb[i * bcount : (i + 1) * bcount])
        in_maps.append({"x": xs, "wts": wts})
    return D, in_maps


def _run(x, alpha, beta, trace=False, **kw):
    x = np.asarray(x, dtype=np.float32)
    assert x.shape == (B, T, C), x.shape
    D, in_maps = make_in_maps(x, alpha, beta)
    nc = _get_module(D)
    res = run_bass_kernel_spmd(nc, in_maps, list(range(N_CORES)), trace=trace, **kw)
    out = np.concatenate(
        [_unswizzle(res.results[i]["y"]).astype(np.float32) for i in range(N_CORES)],
        axis=0,
    )
    return out, res


def kernel(x, alpha, beta):
    return _run(x, alpha, beta)[0]



# revision 18
# speedup vs baseline: 1.0038x; 1.0038x over previous
"""DEMA (double exponential smoothing) Trainium2 Bass kernel.

Math
----
Reference recurrence (per batch b, channel c, over time t):
    s0 = x[0], b0 = x[1] - x[0]
    s_t = a*x_t + (1-a)*(s_{t-1} + b_{t-1})
    b_t = bt*(s_t - s_{t-1}) + (1-bt)*b_{t-1}
    out = [s0, s_1, ..., s_{T-1}]

Eliminating the trend state gives a linear constant-coefficient 2nd-order
recurrence (exact; s_0 = x_0, s_1 = x_1):
    s_t = tau*s_{t-1} - delta*s_{t-2} + b0*x_t + b1*x_{t-1},  t >= 2
    tau = 2 - a - a*bt, delta = 1 - a, b0 = a, b1 = a*((1-a)*(1+bt) - tau)

So out = M @ x along time, where M is lower-triangular with Toeplitz body
M[t,k] = w_{t-k} (w = impulse response, w_j = tau*w_{j-1} - delta*w_{j-2})
plus two special leading columns for the x_0/x_1 initial conditions. The
poles satisfy |lambda| <= sqrt(1-a) < 1, so w decays geometrically and M
is effectively banded: blocking time into 128-chunks, out-block i only
needs input blocks j >= i-D, where D is chosen on host so the dropped
tail is below 1e-8 relative (D=1 for both graded PRNG variants, D=3 for
the worst-case alpha=0.1).

The kernel is a causal blocked convolution on the TensorEngine:
    out_blk[i] = sum_{d=0..min(i,D)} W_d^T @ x_blk[i-d]       (PSUM accum)
with 128x128 bf16 weight blocks W_d (plus special j=0 variants carrying
the initial-condition columns) computed on host in float64 from the
runtime alpha/beta and shipped as a small input tensor. There are no
cross-block dependencies, so the TensorEngine streams back-to-back
matmuls at full clock; PSUM->SBUF eviction alternates ScalarE/VectorE;
x/y move in 512 KiB 128-partition mega-tile DMAs. x and y are stored
in HBM pre-swizzled (host-side, free) to the SBUF tile layout
[b, mega, tl, th*c], so each partition's slice of a mega is a single
contiguous 4 KiB DMA descriptor instead of four 1 KiB ones — ~9% -> ~2%
SDMA per-packet overhead (m2s/s2m bus + 32 B/descriptor metadata),
worth ~3-5% of the DMA-bound runtime on HW (cost-model-neutral).

Precision: the accuracy gate is rel<2e-2 global. Everything on the
device side is bf16 (x cast host-side, y upcast host-side, fp32 PSUM
accumulate): measured rel err 3.2e-3, 6x under the gate, while bf16
runs the PE at 1 cyc/column (4x fp32) and halves HBM traffic to
32 MB/core. That makes the kernel DMA-bound: cost-model timeline sim
(calibrated +1.4% vs HW on the fp32 version) shows the DMA engines
busy 93.9 us back-to-back with ~2 us first-byte latency at the head
and ~1.5 us completion at the tail => ~98 us/core, i.e. ~95% of the
b16 HBM roofline (~358 GB/s/NC). fp8 I/O would halve traffic again
but its 3.6% quantization noise blows the gate; predictive/subsampled
output compression founders on partition-packing (half-partition DMAs
run at half bandwidth, evictions cost by free-dim, matmuls by N).

Sharding: batch 32 -> 4 per core across 8 cores (data parallel; the
recurrence is independent per (b, c)).
"""

import ml_dtypes
import numpy as np

import concourse.bacc as bacc
import concourse.bass as bass
import concourse.mybir as mybir
from concourse import tile
from concourse.bass_utils import run_bass_kernel_spmd

N_CORES = 8
P = 128            # SBUF partitions == time-block length
B, T, C = 32, 4096, 512
BC = B // N_CORES  # batches per core
NBLK = T // P      # 32 time blocks
MEGA = 4           # time blocks per DMA mega-tile (4*128*512*2B = 512 KiB)

_F32 = mybir.dt.float32
# bf16 end-to-end: the accuracy gate is rel<2e-2 global; bf16 I/O +
# bf16 matmul (fp32 PSUM accumulate) lands ~1e-3, with 4x the PE rate
# of fp32 (1 cyc/col vs 4) and half the HBM traffic. x is cast to bf16
# on the host (free), y is written bf16 and upcast on the host.
_MM_DT = mybir.dt.bfloat16


def _host_weights(a: float, bt: float, tol: float = 1e-8):
    """Impulse response + IC columns -> (D, wts[2*(D+1), 128, 128]) lhsT-layout."""
    tau = 2.0 - a - a * bt
    delta = 1.0 - a
    b0 = a
    b1 = a * ((1.0 - a) * (1.0 + bt) - tau)
    n = T
    w = np.zeros(n)
    c0 = np.zeros(n)
    c1 = np.zeros(n)
    w[0] = b0
    w[1] = tau * b0 + b1
    c0[0] = 1.0
    c1[1] = 1.0
    for j in range(2, n):
        w[j] = tau * w[j - 1] - delta * w[j - 2]
        c0[j] = tau * c0[j - 1] - delta * c0[j - 2]
        c1[j] = tau * c1[j - 1] - delta * c1[j - 2] + (b1 if j == 2 else 0.0)
    wnorm = max(np.sqrt((w ** 2).sum()), 1.0)
    D = NBLK - 1
    for d in range(NBLK):
        tail = np.sqrt(
            (w[P * d + 1 :] ** 2).sum()
            + (c0[P * (d + 1) :] ** 2).sum()
            + (c1[P * (d + 1) :] ** 2).sum()
        )
        if tail <= tol * wnorm:
            D = d
            break
    # lhsT layout [k, t]: out[t, n] = sum_k W[k, t] * x[k, n]
    wts = np.zeros((2 * (D + 1), P, P), np.float64)
    kk = np.arange(P)[:, None]
    tt = np.arange(P)[None, :]
    for d in range(D + 1):
        lag = P * d + tt - kk          # [k, t] lag matrix
        Tm = np.where((lag >= 0) & (lag < n), w[np.clip(lag, 0, n - 1)], 0.0)
        Sm = Tm.copy()
        Sm[0, :] = c0[P * d : P * d + P]
        Sm[1, :] = c1[P * d : P * d + P]
        wts[2 * d] = Tm
        wts[2 * d + 1] = Sm
    return D, wts


def _build(D, bcount=BC, t_len=T, c_len=C, mega=MEGA, xbufs=8, obufs=6,
           psbufs=8):
    """Build + compile the per-core SPMD module for diagonal depth D."""
    MEGA = mega
    nblk = t_len // P
    nmega = nblk // MEGA
    nw = 2 * (D + 1)
    nc = bacc.Bacc("TRN2", target_bir_lowering=False, debug=False)
    # x/y live in HBM pre-swizzled to the SBUF tile layout
    # [b, mega, partition(=tl), th*c] so every partition's mega-slice is
    # one contiguous 4 KiB run (one DMA descriptor instead of four 1 KiB
    # ones — ~3-5% better SDMA packet efficiency on HW; the host does the
    # (free) transpose). th = block-within-mega, tl = t % 128.
    x = nc.dram_tensor(
        "x", [bcount, nblk // MEGA, P, MEGA * c_len], _MM_DT, kind="ExternalInput"
    )
    wd = nc.dram_tensor("wts", [nw, P, P], _MM_DT, kind="ExternalInput")
    y = nc.dram_tensor(
        "y", [bcount, nblk // MEGA, P, MEGA * c_len], _MM_DT, kind="ExternalOutput"
    )

    xbufs = max(xbufs, (D + MEGA - 1) // MEGA + 2)
    with tile.TileContext(nc) as tc:
        with (
            tc.tile_pool(name="wpool", bufs=1) as wpool,
            tc.tile_pool(name="xpool", bufs=xbufs) as xpool,
            tc.tile_pool(name="psum", bufs=psbufs, space="PSUM") as pspool,
            tc.tile_pool(name="opool", bufs=obufs) as opool,
        ):
            # weights go on the ACT HWDGE ring so their descriptor-gen
            # overlaps the first x-block DMAs on the SP ring
            wt = wpool.tile([P, nw * P], _MM_DT)
            nc.scalar.dma_start(
                wt[:].rearrange("k (m t) -> k m t", m=nw),
                wd[:].rearrange("m k t -> k m t"),
            )

            xmega: dict = {}
            for b in range(bcount):
                for mg in range(nmega):
                    xm = xpool.tile([P, MEGA * c_len], _MM_DT, tag="xm")
                    xmega[(b, mg)] = xm
                    src = x[b, mg]
                    if b == 0 and mg == 0:
                        # startup fast path: per-block DMAs so the first
                        # matmul only waits for 128 KiB, not the full mega;
                        # alternate HWDGE rings to halve the descriptor-gen
                        # chain ahead of the first matmuls
                        for blk in range(MEGA):
                            eng = nc.sync if blk % 2 == 0 else nc.scalar
                            eng.dma_start(
                                xm[:, blk * c_len : (blk + 1) * c_len],
                                src[:, blk * c_len : (blk + 1) * c_len],
                            )
                    else:
                        nc.sync.dma_start(xm[:], src)
                    om = opool.tile([P, MEGA * c_len], _MM_DT, tag="om")
                    last = b == bcount - 1 and mg == nmega - 1
                    for blk in range(MEGA):
                        i = mg * MEGA + blk
                        ps = pspool.tile([P, c_len], _F32, tag="ps")
                        dmax = min(i, D)
                        for nd, d in enumerate(range(dmax, -1, -1)):
                            j = i - d
                            wsl = 2 * d + (1 if j == 0 else 0)
                            rhs_m = xmega[(b, j // MEGA)]
                            rhs = rhs_m[:, (j % MEGA) * c_len : (j % MEGA + 1) * c_len]
                            nc.tensor.matmul(
                                ps[:],
                                wt[:, wsl * P : (wsl + 1) * P],
                                rhs,
                                start=(nd == 0),
                                stop=(nd == dmax),
                            )
                        dst = om[:, blk * c_len : (blk + 1) * c_len]
                        if i % 2 == 0:
                            nc.scalar.copy(dst, ps[:])
                        else:
                            nc.vector.tensor_copy(dst, ps[:])
                        if last:
                            # tail fast path: store each block right after
                            # its eviction so the final store is 128 KiB
                            nc.scalar.dma_start(
                                y[b, mg][:, blk * c_len : (blk + 1) * c_len], dst
                            )
                    if not last:
                        nc.scalar.dma_start(y[b, mg], om[:])
    nc.compile()
    return nc


_MODULE_CACHE: dict = {}


def _get_module(D, **kw):
    key = (D, tuple(sorted(kw.items())))
    if key not in _MODULE_CACHE:
        _MODULE_CACHE[key] = _build(D, **kw)
    return _MODULE_CACHE[key]


NMEGA = NBLK // MEGA


def _swizzle(xb):
    """[bc, T, C] -> [bc, nmega, P(tl), MEGA*C] (contiguous 4 KiB rows)."""
    v = xb.reshape(xb.shape[0], NMEGA, MEGA, P, C).transpose(0, 1, 3, 2, 4)
    return np.ascontiguousarray(v).reshape(xb.shape[0], NMEGA, P, MEGA * C)


def _unswizzle(ys):
    """Inverse of _swizzle: [bc, nmega, P, MEGA*C] -> [bc, T, C]."""
    v = ys.reshape(ys.shape[0], NMEGA, P, MEGA, C).transpose(0, 1, 3, 2, 4)
    return np.ascontiguousarray(v).reshape(ys.shape[0], T, C)


def make_in_maps(x, alpha, beta, bcount=BC, n_cores=N_CORES):
    a = float(np.asarray(alpha).reshape(-1)[0])
    bt = float(np.asarray(beta).reshape(-1)[0])
    D, wts = _host_weights(a, bt)
    wts = np.ascontiguousarray(wts.astype(ml_dtypes.bfloat16))
    xb = x.astype(ml_dtypes.bfloat16)
    in_maps = []
    for i in range(n_cores):
        xs = _swizzle(xb[i * bcount : (i + 1) * bcount])
        in_maps.append({"x": xs, "wts": wts})
    return D, in_maps


def _run(x, alpha, beta, trace=False, **kw):
    x = np.asarray(x, dtype=np.float32)
    assert x.shape == (B, T, C), x.shape
    D, in_maps = make_in_maps(x, alpha, beta)
    nc = _get_module(D)
    res = run_bass_kernel_spmd(nc, in_maps, list(range(N_CORES)), trace=trace, **kw)
    out = np.concatenate(
        [_unswizzle(res.results[i]["y"]).astype(np.float32) for i in range(N_CORES)],
        axis=0,
    )
    return out, res


def kernel(x, alpha, beta):
    return _run(x, alpha, beta)[0]

